# revision 9
# baseline (speedup 1.0000x reference)
"""Pointer-generator extended-vocab log-softmax (segment_reduce) on 8 Trainium2 cores.

Strategy: one batch row per NeuronCore (B=8, data parallel). The one-hot
projection matmuls in the reference are sparse scatters driven by the tiny
idx tensors, so the kernel never touches the 2x [B,256,16256] one-hot inputs.
Host-side numpy turns the indices into small index-code vectors; the device
streams gen_score and the output in fp16 (8.2 MB each per core):

  out[t, v<V]   = log(exp(gen[t,v]) + exp(c1[t,v]) + exp(c2[t,v])) - log Z[t]
  out[t, V+s]   = log(sum_j exp(cp[t,j])[idx[j]==V+s]) - log Z[t]   (else -1e20)
  Z[t]          = sum_v exp(gen) + sum_n exp(c1ext) + sum_n exp(c2ext)

where c(src)[t,v] = sum_{j: idx(src)[j]==v, v!=0} cp(src)[t,j] is nonzero on at
most 512 "touched" columns (union U). Untouched columns contribute exp(0)=1,
handled by a per-row constant and a bias of 2/Z in the main pass. Touched
columns are computed densely as [256, |U|<=512] via tiny fp16 PE matmuls
against 0/1 matrices built on-chip and written to a small side output that the
host scatters over the final array; that path runs on ACT with exact Exp/Ln.

The dominant per-element work runs on the DVE in 4x (2-byte) perf mode using
Schraudolph-style bit tricks, freeing the ACT engine and halving HBM traffic:
  pass A: e~ = bitcast_f16(int16(g*1024/ln2 + B16)) ~ exp(g), with a second
          bypass tensor_scalar whose accum_out yields the Z row-sums;
  pass B: v = e~*(2^22/Z) + 2*(2^22/Z)  (fp16, comfortably normal-range),
          out = bitcast_i16(v)*ln2/1024 - D  ~ log(v) - 22*ln2.
Max elementwise log error ~0.05; both sawtooths are mean-centered so the
Z sums and the norm-relative error stay ~1e-3, far inside the 2e-2 gate.
"""

import numpy as np

import concourse.bass as bass
import concourse.bacc as bacc
import concourse.mybir as mybir
from concourse.tile import TileContext
from concourse.bass_utils import run_bass_kernel_spmd

B, TDEC, V = 8, 256, 16000
T = 256                  # T1 == T2 (copy-source length)
NOOV = 256               # vocab_size_oov - V
VOOV = V + NOOV
GPAD = 512               # padded |U|; T1+T2 = 512 so never overflows
NEG = np.float32(-1e20)
P = 128
# tapered gen chunks: small first (out-stream starts sooner after Z) and
# small last (shorter phase-A -> Z transition, smaller final straggler)
CHUNKS = [(0, 2000), (2000, 4000), (6000, 4000), (10000, 4000), (14000, 2000)]
NCHUNK = len(CHUNKS)     # 5
NCORES = 8

# fp16 packed small input columns: [cp1T | cp2T | gath]
HOFF_CPT = (0, TDEC)
HOFF_GATH = 2 * TDEC
SMALLH_W = 2 * TDEC + GPAD              # 1024
# f32 packed small input columns: [zb | wpos1 | wpos2 | mpos1 | mpos2]
SOFF_ZB = 0
SOFF_WPOS = (1, 2)
SOFF_MPOS = (3, 4)
SMALLS_W = 5

_LN2 = float(np.log(2.0))
C_LOG = _LN2 / 1024.0                   # fp16 bits -> ln slope
# ln(v) ~ bits(v)*C_LOG - D_LOG (mean-centered sawtooth); per-row -logZ folds in
D_LOG = 15.0 * _LN2 - 0.0397

F32 = mybir.dt.float32
F16 = mybir.dt.float16
F8 = mybir.dt.float8e4
I16 = mybir.dt.int16
AF = mybir.ActivationFunctionType
AX = mybir.AxisListType
ALU = mybir.AluOpType

# The small exact path alternates Exp and Ln on the scalar engine, and the
# act-table-load pass greedily picks the first table set containing each
# func -- thrashing ~1.3us per switch. One act_info set
# ("natural_log_exp_and_others") holds BOTH funcs; hide Exp/Ln from every
# other set (order/indices preserved so act_func_set_id stays aligned with
# act_info.json) so all Exp/Ln activations share one resident table.
_orig_get_tables = bacc.get_activation_tables


def _combined_exp_ln_tables(module_arch):
    tabs = _orig_get_tables(module_arch)
    both = {n for n, s in tabs.items() if AF.Exp in s and AF.Ln in s}
    if both:
        keep = next(iter(both))
        tabs = {
            n: (s if n == keep else (s - {AF.Exp, AF.Ln}))
            for n, s in tabs.items()
        }
    return tabs


bacc.get_activation_tables = _combined_exp_ln_tables


def _build_kernel() -> bass.Bass:
    nc = bacc.Bacc(trn_type="TRN2", num_devices=NCORES)

    gen = nc.dram_tensor("gen", [TDEC, V], F8, kind="ExternalInput")
    smallh = nc.dram_tensor("smallh", [TDEC, SMALLH_W], F16, kind="ExternalInput")
    smalls = nc.dram_tensor("smalls", [TDEC, SMALLS_W], F32, kind="ExternalInput")

    out_main = nc.dram_tensor("out_main", [TDEC, V], F16, kind="ExternalOutput")
    # [:, :GPAD] = touched-column values, [:, GPAD:] = OOV block
    out_small = nc.dram_tensor("out_small", [TDEC, GPAD + NOOV], F16,
                               kind="ExternalOutput")

    with TileContext(nc) as tc:
        with (
            tc.tile_pool(name="big", bufs=4) as big,
            tc.tile_pool(name="emain", bufs=1) as emain,
            tc.tile_pool(name="small", bufs=1) as small,
            tc.tile_pool(name="psum", bufs=1, space="PSUM") as psum,
        ):
            # ---- packed small-input DMAs per 128-row tile ----
            smh, sms = [], []
            for k in range(2):
                th = small.tile([P, SMALLH_W], F16, tag=f"smh{k}", name=f"smh{k}")
                nc.sync.dma_start(th, smallh[k * P:(k + 1) * P, :])
                smh.append(th)
                ts_ = small.tile([P, SMALLS_W], F32, tag=f"sms{k}", name=f"sms{k}")
                nc.sync.dma_start(ts_, smalls[k * P:(k + 1) * P, :])
                sms.append(ts_)

            def cpt_sb(s, k):
                return smh[k][:, HOFF_CPT[s]:HOFF_CPT[s] + TDEC]

            # ---- build W [j,u]=(wpos[j]==u) and M [j,s]=(mpos[j]==s) on chip ----
            iot_i = small.tile([P, GPAD], mybir.dt.int32, tag="iot_i", name="iot_i")
            nc.gpsimd.iota(iot_i, [[1, GPAD]], channel_multiplier=0)
            iot = small.tile([P, GPAD], F32, tag="iot", name="iot")
            nc.vector.tensor_copy(iot, iot_i)
            w_t = [[None] * 2 for _ in range(2)]
            m_t = [[None] * 2 for _ in range(2)]
            for s in range(2):
                for k in range(2):
                    wt = small.tile([P, GPAD], F16, tag=f"w{s}{k}", name=f"w{s}{k}")
                    code = sms[k][:, SOFF_WPOS[s]:SOFF_WPOS[s] + 1]
                    nc.vector.tensor_scalar(out=wt, in0=iot, scalar1=code,
                                            scalar2=None, op0=ALU.is_equal)
                    w_t[s][k] = wt
                    mt = small.tile([P, NOOV], F16, tag=f"m{s}{k}", name=f"m{s}{k}")
                    code = sms[k][:, SOFF_MPOS[s]:SOFF_MPOS[s] + 1]
                    nc.vector.tensor_scalar(out=mt, in0=iot[:, :NOOV], scalar1=code,
                                            scalar2=None, op0=ALU.is_equal)
                    m_t[s][k] = mt

            # Z addends strip: [:, :NCHUNK] gen-chunk partials, then
            # esc rowsums (2), acc rowsum, zb  ->  9 columns total
            pacc = []
            for m in range(2):
                tp = small.tile([P, NCHUNK + 4], F32, tag=f"pacc{m}",
                                name=f"pacc{m}")
                nc.vector.tensor_copy(tp[:, NCHUNK + 3:NCHUNK + 4],
                                      sms[m][:, SOFF_ZB:SOFF_ZB + 1])
                pacc.append(tp)

            # ---- ACT Exp block: exp(cpT) for the OOV-bucket matmuls ----
            ecp = [[None] * 2 for _ in range(2)]
            for s in range(2):
                for k in range(2):
                    te = small.tile([P, TDEC], F16, tag=f"ecp{s}{k}",
                                    name=f"ecp{s}{k}")
                    nc.scalar.activation(te, cpt_sb(s, k), AF.Exp)
                    ecp[s][k] = te

            # ---- pass A + small path, per t-tile ----
            # ACT queue order: ecp (above) -> m0 gen Exps -> m0 esc/gath ->
            # m1 gen Exps -> m1 esc/gath, so the big Exps start as soon as
            # the first gen chunk lands and Z[m] closes right after its
            # last chunk. The Z-free half of pass B (v = e+2) rides along
            # on DVE per chunk; only the fast-log waits for Z.
            e_tiles = [[None] * NCHUNK for _ in range(2)]
            esc = [[None] * 2 for _ in range(2)]
            os_sb = [None] * 2
            mask_sb = [None] * 2
            accc_sb = [None] * 2
            for m in range(2):
                mm = slice(m * P, (m + 1) * P)

                # stream gen (fp8), ACT Exp (exact) + free Z accum, then +2
                for c, (off, w) in enumerate(CHUNKS):
                    gt = big.tile([P, w], F8, tag=f"g{w}", name=f"g{m}{c}",
                                  bufs=(4 if w == 4000 else 3))
                    nc.sync.dma_start(gt, gen[mm, off:off + w])
                    et = emain.tile([P, w], F16, tag=f"e{m}{c}", name=f"e{m}{c}")
                    nc.scalar.activation(et, gt, AF.Exp,
                                         accum_out=pacc[m][:, c:c + 1])
                    nc.vector.tensor_scalar(out=et, in0=et, scalar1=2.0,
                                            scalar2=None, op0=ALU.add)
                    e_tiles[m][c] = et

                # touched-column (SC) matmuls -> exp, exp(gath)
                for s in range(2):
                    pt = psum.tile([P, GPAD], F32, tag=f"scp{m}{s}",
                                   name=f"scp{m}{s}")
                    nc.tensor.matmul(pt, lhsT=cpt_sb(s, 0)[:, mm], rhs=w_t[s][0],
                                     start=True, stop=False)
                    nc.tensor.matmul(pt, lhsT=cpt_sb(s, 1)[:, mm], rhs=w_t[s][1],
                                     start=False, stop=True)
                    te = small.tile([P, GPAD], F32, tag=f"esc{m}{s}",
                                    name=f"esc{m}{s}")
                    nc.scalar.activation(te, pt, AF.Exp,
                                         accum_out=pacc[m][:, NCHUNK + s:
                                                           NCHUNK + s + 1])
                    esc[m][s] = te
                ot = small.tile([P, GPAD], F32, tag=f"os{m}", name=f"os{m}")
                nc.scalar.activation(ot, smh[m][:, HOFF_GATH:HOFF_GATH + GPAD],
                                     AF.Exp)
                os_sb[m] = ot

                # OOV-bucket matmuls (acc) + derived DVE tensors
                ap = psum.tile([P, NOOV], F32, tag=f"accp{m}", name=f"accp{m}")
                steps = [(s, k) for s in range(2) for k in range(2)]
                for i, (s, k) in enumerate(steps):
                    nc.tensor.matmul(ap, lhsT=ecp[s][k][:, mm], rhs=m_t[s][k],
                                     start=(i == 0), stop=(i == len(steps) - 1))
                nc.vector.reduce_sum(out=pacc[m][:, NCHUNK + 2:NCHUNK + 3],
                                     in_=ap, axis=AX.X)
                tmask = small.tile([P, NOOV], mybir.dt.uint8, tag=f"mask{m}",
                                   name=f"mask{m}")
                nc.vector.tensor_scalar(out=tmask, in0=ap, scalar1=0.0,
                                        scalar2=None, op0=ALU.is_gt)
                mask_sb[m] = tmask
                tacc = small.tile([P, NOOV], F32, tag=f"accc{m}", name=f"accc{m}")
                nc.vector.tensor_scalar_max(out=tacc, in0=ap, scalar1=1e-30)
                accc_sb[m] = tacc
                # tu = exp(gath) + esc1 + esc2 on DVE
                tu = os_sb[m]
                nc.vector.tensor_add(tu, tu, esc[m][0])
                nc.vector.tensor_add(tu, tu, esc[m][1])

            for m in range(2):
                mm = slice(m * P, (m + 1) * P)

                # ---- Z -> 1/Z (small path) and -(logZ + D) (pass B) ----
                tz = small.tile([P, 1], F32, tag=f"z{m}", name=f"z{m}")
                nc.vector.reduce_sum(out=tz, in_=pacc[m], axis=AX.X)
                trcp = small.tile([P, 1], F32, tag=f"rcp{m}", name=f"rcp{m}")
                nc.vector.reciprocal(trcp, tz)
                tlz = small.tile([P, 1], F32, tag=f"lz{m}", name=f"lz{m}")
                nc.scalar.activation(tlz, tz, AF.Ln)
                nlz = small.tile([P, 1], F32, tag=f"nlz{m}", name=f"nlz{m}")
                nc.vector.tensor_scalar(out=nlz, in0=tlz, scalar1=-1.0,
                                        scalar2=-D_LOG, op0=ALU.mult,
                                        op1=ALU.add)

                # ---- small outputs (exact ACT Ln) ----
                osh = small.tile([P, GPAD + NOOV], F16, tag=f"osh{m}",
                                 name=f"osh{m}")
                nc.scalar.activation(osh[:, :GPAD], os_sb[m], AF.Ln, scale=trcp)
                tl2 = small.tile([P, NOOV], F16, tag=f"tl2{m}", name=f"tl2{m}")
                nc.scalar.activation(tl2, accc_sb[m], AF.Ln, scale=trcp)
                tneg = small.tile([P, NOOV], F16, tag=f"neg{m}", name=f"neg{m}")
                # finite fp16 sentinel; host maps anything < -1e4 back to -1e20
                nc.vector.memset(tneg, -60000.0)
                nc.vector.select(osh[:, GPAD:], mask_sb[m], tl2, tneg)
                # split across 4 transfers so the ~400KB lands on several
                # DMA rings instead of serializing on one engine
                for q in range(4):
                    nc.sync.dma_start(
                        out_small[m * P + q * (P // 4):
                                  m * P + (q + 1) * (P // 4), :],
                        osh[q * (P // 4):(q + 1) * (P // 4), :])

                # ---- pass B tail (DVE): out = bits(e+2)*c - logZ - D ----
                for c, (off, w) in enumerate(CHUNKS):
                    ef = e_tiles[m][c]
                    nc.vector.tensor_scalar(out=ef, in0=ef.bitcast(I16),
                                            scalar1=C_LOG, scalar2=nlz,
                                            op0=ALU.mult, op1=ALU.add)
                    nc.sync.dma_start(out_main[mm, off:off + w], ef)

    nc.compile()
    return nc


_NC_CACHE: list = []


def _get_nc() -> bass.Bass:
    if not _NC_CACHE:
        _NC_CACHE.append(_build_kernel())
    return _NC_CACHE[0]


def _host_prep(gen_b, cp1_b, cp2_b, idx1_b, idx2_b):
    """Build one core's packed input tensors from one batch row."""
    idx1 = idx1_b.astype(np.int64)
    idx2 = idx2_b.astype(np.int64)
    inv1 = idx1 < V
    inv2 = idx2 < V

    U = np.unique(np.concatenate([idx1[inv1 & (idx1 != 0)],
                                  idx2[inv2 & (idx2 != 0)]]))
    G = len(U)

    smallh = np.zeros((TDEC, SMALLH_W), np.float16)
    smallh[:, HOFF_CPT[0]:HOFF_CPT[0] + TDEC] = cp1_b.T
    smallh[:, HOFF_CPT[1]:HOFF_CPT[1] + TDEC] = cp2_b.T
    if G:
        smallh[:, HOFF_GATH:HOFF_GATH + G] = gen_b[:, U]

    smalls = np.zeros((TDEC, SMALLS_W), np.float32)
    for s, (idx, inv) in enumerate(((idx1, inv1), (idx2, inv2))):
        wpos = np.full(T, -1, np.int64)
        sel = inv & (idx != 0)
        if sel.any():
            wpos[sel] = np.searchsorted(U, idx[sel])
        smalls[:, SOFF_WPOS[s]] = wpos.astype(np.float32)
        mpos = np.full(T, -1, np.int64)
        sel = idx >= V
        if sel.any():
            mpos[sel] = idx[sel] - V
        smalls[:, SOFF_MPOS[s]] = mpos.astype(np.float32)

    cnt_inv = int(inv1.sum()) + int(inv2.sum())
    smalls[:, SOFF_ZB] = np.float32(2.0 * (V - GPAD) + cnt_inv)

    import ml_dtypes
    in_map = {
        "gen": np.ascontiguousarray(gen_b).astype(ml_dtypes.float8_e4m3),
        "smallh": smallh,
        "smalls": smalls,
    }
    return in_map, U


def kernel(**inputs) -> np.ndarray:
    gen_score = np.asarray(inputs["gen_score"], np.float32)
    cp_score1 = np.asarray(inputs["cp_score1"], np.float32)
    cp_score2 = np.asarray(inputs["cp_score2"], np.float32)
    idx_oov1 = np.asarray(inputs["idx_oov1"])
    idx_oov2 = np.asarray(inputs["idx_oov2"])

    in_maps, metas = [], []
    for b in range(B):
        im, U = _host_prep(gen_score[b], cp_score1[b], cp_score2[b],
                           idx_oov1[b], idx_oov2[b])
        in_maps.append(im)
        metas.append(U)

    nc = _get_nc()
    res = run_bass_kernel_spmd(nc, in_maps, core_ids=list(range(NCORES)))

    out = np.empty((B, TDEC, VOOV), np.float32)
    for b in range(B):
        r = res.results[b]
        ob = out[b]
        ob[:, :V] = r["out_main"]
        oov = np.asarray(r["out_small"][:, GPAD:], np.float32)
        ob[:, V:] = np.where(oov < -1e4, NEG, oov)
        U = metas[b]
        if len(U):
            ob[:, U] = r["out_small"][:, :len(U)]
    return out
